# revision 37
# baseline (speedup 1.0000x reference)
"""Long convolution (FFT conv + residual) on 8 Trainium2 NeuronCores.

Math (identical to the reference):
  out[b,l,h] = x[b,l,h] + sum_{s<=l} x[b,s,h]*filt[h,l-s]
computed as a zero-padded circular convolution with an FFT of size
N = 2L = 8192. The residual is folded into the filter on the host
(filt[h,0] += 1), so the device computes only the convolution.

Sharding: channel-parallel over the hidden dim -- 128 of the 1024
channels per core, no inter-core communication. Two real sequences
(batches 2p, 2p+1) are packed as one complex sequence z = x[2p] +
i*x[2p+1]; the filter is real, so Re/Im of the inverse transform are
the two convolutions.

FFT(8192) = four-step Cooley-Tukey, 8192 = 64 x 128, as TensorEngine
matmuls (bf16 in, fp32 PSUM) with NO transposes anywhere:
  step1  per-seq data slice is the matmul *stationary*:
         psum_s[n2, :] = A_s^T @ [W64re|W64im] (+ conjugate partner)
         which lands B^T with n2 on partitions, ready for step2.
  tw     C^T = B^T * exp(-2pi i n2 k1/8192)   (VectorE, bf16 2x mode)
  step2  X[k2,(s,k1)] = W128^T @ C^T          (batched over sequences)
  pw     P = X * Kf on GpSimdE (filter FFT precomputed on host)
  invA   per-seq P slice as stationary: Q^T_s = P_s^T @ [WAre|WAim]
  twj    R^T = Q^T * exp(+2pi i n2' k1/8192)  (VectorE)
  invC   cat stationary [WCre|WCim] yields Yre (batch 2p) on psum
         partitions 0:32 and Yim (batch 2p+1) on 32:64 in one pair of
         accumulating matmuls; first 4096 samples only.
ScalarE handles all PSUM->SBUF casts. Host packs x into the per-core
DMA-friendly layout U[p, n1, h', n2] = x[2p(+1), 128*n1+n2, h0+h']
(bf16) and unpacks O[b, n1, h', n2] -> out[b, 128*n1+n2, h].

If the Trainium path is unavailable (no axon NeuronCores), kernel()
falls back to an exact scipy/numpy FFT implementation.
"""

import sys

sys.path.insert(0, "/opt/trn_rl_repo")

import numpy as np
import ml_dtypes

B, L, H = 4, 4096, 1024
NCORES = 8
HSH = H // NCORES  # 128 channels per core
N = 2 * L  # 8192
N1, N2 = 64, 128  # N = N1 * N2
S = 16  # sequences (h' channels) per tile
BF = ml_dtypes.bfloat16

_cache = {}


def _consts():
    n1 = np.arange(32)[:, None]
    k1 = np.arange(64)[None, :]
    W1 = np.exp(-2j * np.pi * (n1 * k1) / 64.0)  # [32,64]
    n2 = np.arange(128)[:, None]
    k2 = np.arange(128)[None, :]
    W2 = np.exp(-2j * np.pi * (n2 * k2) / 128.0)  # [128,128] lhsT step2
    WA = np.exp(2j * np.pi * (n2 * k2) / 128.0)  # [128,128] rhs of invA
    k1c = np.arange(64)[:, None]
    m1 = np.arange(32)[None, :]
    WC = np.exp(2j * np.pi * (k1c * m1) / 64.0)  # [64,32] lhsT invC
    # fwd twiddle (transposed layout) Tt[n2,k1] = exp(-2pi i n2*k1/8192)
    Tt = np.exp(-2j * np.pi * np.outer(np.arange(128), np.arange(64)) / 8192.0)
    # inv twiddle (transposed layout) Tj[k1,n2'] = exp(+2pi i n2'*k1/8192)
    Tj = np.exp(2j * np.pi * np.outer(np.arange(64), np.arange(128)) / 8192.0)

    def b(a):
        return np.ascontiguousarray(a).astype(BF)

    def f(a):
        return np.ascontiguousarray(a).astype(np.float32)

    c = {}
    # step1 (data as stationary): rhs cats [32,128]
    c["w64cat"] = b(np.concatenate([W1.real, W1.imag], axis=1))
    c["w64cat2"] = b(np.concatenate([-W1.imag, W1.real], axis=1))
    c["w64catf"] = b(np.concatenate([W1.real / N, W1.imag / N], axis=1))
    # step2 stationaries [128,128]
    c["s2re"] = b(W2.real)
    c["s2im"] = b(W2.imag)
    c["s2imneg"] = b(-W2.imag)
    # invA (data as stationary): rhs cats [128,256]
    c["wacat"] = b(np.concatenate([WA.real, WA.imag], axis=1))
    c["wacat2"] = b(np.concatenate([-WA.imag, WA.real], axis=1))
    # invC cat stationaries [64,64]: rows 0:32 -> Yre, 32:64 -> Yim
    c["sccat"] = b(np.concatenate([WC.real, WC.imag], axis=1))
    c["sccat2"] = b(np.concatenate([-WC.imag, WC.real], axis=1))
    # twiddles, tiled S times along seqs
    c["twfre"] = b(np.tile(Tt.real, (1, S)))  # [128, 64*S]
    c["twfim"] = b(np.tile(Tt.imag, (1, S)))
    c["twjre"] = b(np.tile(Tj.real, (1, S)))  # [64, 128*S]
    c["twjim"] = b(np.tile(Tj.imag, (1, S)))
    return c


def _build(reps=1):
    import concourse.mybir as mybir
    import concourse.tile as tile
    from concourse import bacc

    bf16 = mybir.dt.bfloat16
    f32 = mybir.dt.float32

    nc = bacc.Bacc("TRN2", target_bir_lowering=False, debug=False, num_devices=NCORES)

    ure_d = nc.dram_tensor("ure", [2, 32, HSH, 128], bf16, kind="ExternalInput").ap()
    uim_d = nc.dram_tensor("uim", [2, 32, HSH, 128], bf16, kind="ExternalInput").ap()
    # all constants + the host-computed filter FFT ride in ONE packed
    # input tensor -- fewer per-dispatch args on the axon tunnel
    co = _consts()
    layout, col = {}, 0
    for nm in sorted(co):
        r, w = co[nm].shape
        layout[nm] = (r, col, w)
        col += w
    layout["kfre"] = (128, col, HSH * 64)
    col += HSH * 64
    layout["kfim"] = (128, col, HSH * 64)
    col += HSH * 64
    cpack_d = nc.dram_tensor("cpack", [128, col], bf16, kind="ExternalInput").ap()
    _build.layout = (dict(layout), col)
    oc_d = nc.dram_tensor("oc", [4, 32, HSH, 128], bf16, kind="ExternalOutput").ap()

    NT = HSH // S  # tiles per (pair / filter) pass

    with tile.TileContext(nc) as tc:
        with (
            tc.tile_pool(name="consts", bufs=1) as cpool,
            tc.tile_pool(name="kf", bufs=1) as kfpool,
            tc.tile_pool(name="work", bufs=2) as wp,
            # four per-stage double-buffered tags -> 8 banks total
            tc.tile_pool(name="psmm", bufs=2, space="PSUM") as pmm,
            tc.tile_pool(name="dscratch", bufs=2, space="DRAM") as dsp,
        ):
            # intermediate timing-reps write to scratch, not the real output
            oc_reps = [
                (dsp.tile([4, 32, HSH, 128], f32, name=f"ocs{r}")[:]
                 if r < reps - 1 else oc_d)
                for r in range(reps)
            ]
            sb = {}
            for nm in sorted(co):
                r, c0, w = layout[nm]
                t = cpool.tile([r, w], bf16, name=f"c_{nm}")
                nc.sync.dma_start(t[:], cpack_d[0:r, c0 : c0 + w])
                sb[nm] = t

            # resident filter FFT [k2=128, h'(128) x k1(64)], host-computed
            kfre = kfpool.tile([128, HSH * 64], bf16, name="kfre")
            kfim = kfpool.tile([128, HSH * 64], bf16, name="kfim")
            r, c0, w = layout["kfre"]
            nc.sync.dma_start(kfre[:], cpack_d[:, c0 : c0 + w])
            r, c0, w = layout["kfim"]
            nc.sync.dma_start(kfim[:], cpack_d[:, c0 : c0 + w])

            def cmul(eng, out_re, out_im, a_re, a_im, b_re, b_im, shape, tag):
                """Elementwise complex multiply via 6 bf16 ops on `eng`."""
                t1 = wp.tile(shape, bf16, tag=f"{tag}1")
                t2 = wp.tile(shape, bf16, tag=f"{tag}2")
                t1v, t2v = t1[:], t2[:]
                if len(a_re.shape) == 3:
                    t1v = t1v.rearrange("p (s k) -> p s k", s=a_re.shape[1])
                    t2v = t2v.rearrange("p (s k) -> p s k", s=a_re.shape[1])
                eng.tensor_mul(t1v, a_re, b_re)
                eng.tensor_mul(t2v, a_im, b_im)
                eng.tensor_sub(out_re, t1v, t2v)
                eng.tensor_mul(t1v, a_re, b_im)
                eng.tensor_mul(t2v, a_im, b_re)
                eng.tensor_add(out_im, t1v, t2v)

            # ---- data passes (filter FFT comes precomputed from host) ----
            # reps>1 repeats the whole workload for timing (idempotent)
            for rep in range(reps):
              oc_t = oc_reps[rep]
              for p in range(2):
                for it in range(NT):
                    are = wp.tile([32, S * 128], bf16, tag="are", bufs=4)
                    aim = wp.tile([32, S * 128], bf16, tag="aim", bufs=4)
                    nc.sync.dma_start(
                        are[:],
                        ure_d[p, :, it * S : (it + 1) * S, :].rearrange(
                            "a b c -> a (b c)"
                        ),
                    )
                    nc.sync.dma_start(
                        aim[:],
                        uim_d[p, :, it * S : (it + 1) * S, :].rearrange(
                            "a b c -> a (b c)"
                        ),
                    )

                    # step1, data slice as stationary:
                    # psum_s[n2, 0:64|64:128] = A_s^T @ [W64re|W64im] + conj
                    btsb = wp.tile([128, S * 128], bf16, tag="btsb", bufs=3)
                    for g in range(S // 4):
                        bt = pmm.tile([128, 512], f32, tag="mm1", bufs=2)
                        for j in range(4):
                            s_ = g * 4 + j
                            osl = slice(j * 128, (j + 1) * 128)
                            lre = are[:, s_ * 128 : (s_ + 1) * 128]
                            lim = aim[:, s_ * 128 : (s_ + 1) * 128]
                            nc.tensor.matmul(bt[:, osl], lre, sb["w64cat"][:], start=True, stop=False)
                            nc.tensor.matmul(bt[:, osl], lim, sb["w64cat2"][:], start=False, stop=True)
                        nc.any.tensor_copy(
                            out=btsb[:, g * 512 : (g + 1) * 512], in_=bt[:]
                        )
                    # fwd twiddle, whole tile in one 6-op pass (DVE)
                    ctre = wp.tile([128, S * 64], bf16, tag="ctre")
                    ctim = wp.tile([128, S * 64], bf16, tag="ctim")
                    v = btsb[:].rearrange("p (s c k) -> p s c k", s=S, c=2, k=64)
                    cmul(
                        nc.vector,
                        ctre[:].rearrange("p (s k) -> p s k", s=S),
                        ctim[:].rearrange("p (s k) -> p s k", s=S),
                        v[:, :, 0, :], v[:, :, 1, :],
                        sb["twfre"][:].rearrange("p (s k) -> p s k", s=S),
                        sb["twfim"][:].rearrange("p (s k) -> p s k", s=S),
                        [128, S * 64], "twf",
                    )

                    # step2: X = W128^T @ C^T  (psum -> bf16 staging)
                    xre_sb = wp.tile([128, S * 64], bf16, tag="xre_sb", bufs=3)
                    xim_sb = wp.tile([128, S * 64], bf16, tag="xim_sb", bufs=3)
                    for c in range(S * 64 // 512):
                        sl = slice(c * 512, (c + 1) * 512)
                        xre = pmm.tile([128, 512], f32, tag="mm2", bufs=4)
                        xim = pmm.tile([128, 512], f32, tag="mm2", bufs=4)
                        nc.tensor.matmul(xre[:], sb["s2re"][:], ctre[:, sl], start=True, stop=False)
                        nc.tensor.matmul(xre[:], sb["s2imneg"][:], ctim[:, sl], start=False, stop=True)
                        nc.tensor.matmul(xim[:], sb["s2im"][:], ctre[:, sl], start=True, stop=False)
                        nc.tensor.matmul(xim[:], sb["s2re"][:], ctim[:, sl], start=False, stop=True)
                        nc.any.tensor_copy(out=xre_sb[:, sl], in_=xre[:])
                        nc.any.tensor_copy(out=xim_sb[:, sl], in_=xim[:])

                    # pointwise P = X * Kf on GpSimd (frees DVE for twiddles)
                    pre = wp.tile([128, S * 64], bf16, tag="pre", bufs=3)
                    pim = wp.tile([128, S * 64], bf16, tag="pim", bufs=3)
                    ksl = slice(it * S * 64, (it + 1) * S * 64)
                    cmul(nc.gpsimd, pre[:], pim[:], xre_sb[:], xim_sb[:],
                         kfre[:, ksl], kfim[:, ksl], [128, S * 64], "pw")

                    # invA, data slice as stationary: Q^T_s = P_s^T @ [WA|..]
                    qtsb = wp.tile([64, S * 256], bf16, tag="qtsb", bufs=4)
                    for g in range(S // 2):
                        qt = pmm.tile([64, 512], f32, tag="mmi", bufs=2)
                        for j in range(2):
                            s_ = g * 2 + j
                            osl = slice(j * 256, (j + 1) * 256)
                            nc.tensor.matmul(qt[:, osl], pre[:, s_ * 64 : (s_ + 1) * 64], sb["wacat"][:], start=True, stop=False)
                            nc.tensor.matmul(qt[:, osl], pim[:, s_ * 64 : (s_ + 1) * 64], sb["wacat2"][:], start=False, stop=True)
                        nc.any.tensor_copy(
                            out=qtsb[:, g * 512 : (g + 1) * 512], in_=qt[:]
                        )
                    # inv twiddle, whole tile in one 6-op pass (DVE)
                    rtre = wp.tile([64, S * 128], bf16, tag="rtre")
                    rtim = wp.tile([64, S * 128], bf16, tag="rtim")
                    v = qtsb[:].rearrange("p (s c n) -> p s c n", s=S, c=2, n=128)
                    cmul(
                        nc.vector,
                        rtre[:].rearrange("p (s n) -> p s n", s=S),
                        rtim[:].rearrange("p (s n) -> p s n", s=S),
                        v[:, :, 0, :], v[:, :, 1, :],
                        sb["twjre"][:].rearrange("p (s n) -> p s n", s=S),
                        sb["twjim"][:].rearrange("p (s n) -> p s n", s=S),
                        [64, S * 128], "twj",
                    )

                    # invC with cat stationaries: one matmul pair yields
                    # Yre on partitions 0:32 and Yim on 32:64
                    for c in range(S * 128 // 512):
                        sl = slice(c * 512, (c + 1) * 512)
                        y = pmm.tile([64, 512], f32, tag="mmi", bufs=2)
                        nc.tensor.matmul(y[:], sb["sccat"][:], rtre[:, sl], start=True, stop=False)
                        nc.tensor.matmul(y[:], sb["sccat2"][:], rtim[:, sl], start=False, stop=True)
                        ysb = wp.tile([64, 512], bf16, tag="ysb", bufs=4)
                        nc.any.tensor_copy(out=ysb[:], in_=y[:])
                        hsl = slice(it * S + c * 4, it * S + (c + 1) * 4)
                        nc.sync.dma_start(
                            oc_t[2 * p, :, hsl, :].rearrange("a b c -> a (b c)"),
                            ysb[0:32, :],
                        )
                        nc.sync.dma_start(
                            oc_t[2 * p + 1, :, hsl, :].rearrange("a b c -> a (b c)"),
                            ysb[32:64, :],
                        )

    nc.compile()
    return nc


def _prep_inputs(x, filt):
    consts = _consts()
    filt2 = filt.copy()
    filt2[:, 0] += 1.0  # fold residual: conv with (filt + delta) = y + u
    # U_all[b, n1, h, n2] = x[b, 128*n1+n2, h]
    u_all = np.ascontiguousarray(
        x.reshape(B, 32, 128, H).transpose(0, 1, 3, 2)
    ).astype(BF)  # [4,32,1024,128]
    # filter FFT on host: full[h, k1 + 64*k2] -> [h, k2, k1]
    try:
        import scipy.fft as _sf

        r = _sf.rfft(filt2, n=N, workers=-1).astype(np.complex64) / np.float32(N)
    except Exception:  # pragma: no cover
        r = (np.fft.rfft(filt2, n=N) / N).astype(np.complex64)
    full = np.concatenate([r, np.conj(r[:, -2:0:-1])], axis=1)  # [H, 8192]
    karr = full.reshape(H, 128, 64)  # [h, k2, k1]
    _get_nc()  # ensure _build.layout exists
    layout, totc = _build.layout
    base = np.zeros((128, totc), BF)
    for nm, arr in consts.items():
        r, c0, w = layout[nm]
        base[0:r, c0 : c0 + w] = arr
    in_maps = []
    for c in range(NCORES):
        h0 = c * HSH
        usl = u_all[:, :, h0 : h0 + HSH, :]  # [4,32,128,128]
        kc = karr[h0 : h0 + HSH].transpose(1, 0, 2)  # [k2, h', k1]
        cp = base.copy()
        r, c0, w = layout["kfre"]
        cp[:, c0 : c0 + w] = kc.real.astype(BF).reshape(128, HSH * 64)
        r, c0, w = layout["kfim"]
        cp[:, c0 : c0 + w] = kc.imag.astype(BF).reshape(128, HSH * 64)
        m = {
            "ure": np.ascontiguousarray(usl[0::2]),
            "uim": np.ascontiguousarray(usl[1::2]),
            "cpack": cp,
        }
        in_maps.append(m)
    return in_maps


def _postprocess(results):
    out = np.empty((B, L, H), np.float32)
    ov = out.reshape(B, 32, 128, H)
    for c in range(NCORES):
        oc = results[c]["oc"]  # [4,32,128,128] = (b, n1, h', n2)
        ov[:, :, :, c * HSH : (c + 1) * HSH] = oc.transpose(0, 1, 3, 2)
    return out


def _get_nc():
    if "nc" not in _cache:
        _cache["nc"] = _build()
    return _cache["nc"]


def _make_sharded(nc):
    """Build a cached jitted shard_map executable for a compiled module."""
    import jax
    from jax.sharding import Mesh, PartitionSpec
    from jax.experimental.shard_map import shard_map
    import concourse.mybir as mybir
    from concourse import bass2jax

    bass2jax.install_neuronx_cc_hook()
    assert nc.dbg_addr is None
    pname = nc.partition_id_tensor.name if nc.partition_id_tensor else None
    in_names, out_names, out_avals, zero_outs = [], [], [], []
    for alloc in nc.m.functions[0].allocations:
        if not isinstance(alloc, mybir.MemoryLocationSet):
            continue
        name = alloc.memorylocations[0].name
        if alloc.kind == "ExternalInput":
            if name != pname:
                in_names.append(name)
        elif alloc.kind == "ExternalOutput":
            out_names.append(name)
            shape = tuple(alloc.tensor_shape)
            dtype = mybir.dt.np(alloc.dtype)
            out_avals.append(jax.core.ShapedArray(shape, dtype))
            zero_outs.append(np.zeros((NCORES * shape[0], *shape[1:]), dtype))
    all_names = in_names + out_names
    if pname is not None:
        all_names = all_names + [pname]

    def _body(*args):
        operands = list(args)
        if pname is not None:
            operands.append(bass2jax.partition_id_tensor())
        outs = bass2jax._bass_exec_p.bind(
            *operands,
            out_avals=tuple(out_avals),
            in_names=tuple(all_names),
            out_names=tuple(out_names),
            lowering_input_output_aliases=(),
            sim_require_finite=True,
            sim_require_nnan=True,
            nc=nc,
        )
        return tuple(outs)

    mesh = Mesh(np.asarray(jax.devices()[:NCORES]), ("core",))
    nin = len(in_names) + len(out_names)
    sharded = jax.jit(
        shard_map(
            _body,
            mesh=mesh,
            in_specs=(PartitionSpec("core"),) * nin,
            out_specs=(PartitionSpec("core"),) * len(out_names),
            check_rep=False,
        ),
        keep_unused=True,
    )
    return sharded, in_names, out_names, mesh, zero_outs


def _build_cal():
    """Do-nothing module (one tiny DMA) used to measure the per-execution
    dispatch floor of the axon PJRT tunnel."""
    import concourse.mybir as mybir
    import concourse.tile as tile
    from concourse import bacc

    nc = bacc.Bacc("TRN2", target_bir_lowering=False, debug=False, num_devices=NCORES)
    xi = nc.dram_tensor("xi", [32, 64], mybir.dt.float32, kind="ExternalInput").ap()
    xo = nc.dram_tensor("xo", [32, 64], mybir.dt.float32, kind="ExternalOutput").ap()
    with tile.TileContext(nc) as tc:
        with tc.tile_pool(name="p", bufs=1) as pool:
            t = pool.tile([32, 64], mybir.dt.float32)
            nc.sync.dma_start(t[:], xi[:])
            nc.sync.dma_start(xo[:], t[:])
    nc.compile()
    return nc


def _marginal_ns(sharded, dev_args, iters=20, reps=5):
    import time
    import jax

    def run_n(n):
        t0 = time.perf_counter()
        res = None
        for _ in range(n):
            res = sharded(*dev_args)
        jax.block_until_ready(res)
        return time.perf_counter() - t0

    run_n(3)  # warmup
    t1 = min(run_n(1) for _ in range(2 * reps))
    tn = min(run_n(1 + iters) for _ in range(reps))
    return (tn - t1) / iters * 1e9


def _get_exec():
    if "exec" not in _cache:
        _cache["exec"] = _make_sharded(_get_nc())
    return _cache["exec"]


REPS = 5


def _get_rep_exec():
    if "rexec" not in _cache:
        _cache["rexec"] = _make_sharded(_build(REPS))
    return _cache["rexec"]


def _get_cal_exec():
    if "cal" not in _cache:
        _cache["cal"] = _make_sharded(_build_cal())
    return _cache["cal"]


def _concat_inputs(in_maps, in_names):
    return [
        np.concatenate([in_maps[c][nm] for c in range(NCORES)], axis=0)
        for nm in in_names
    ]


def _kernel_cpu(x: np.ndarray, filt: np.ndarray) -> np.ndarray:
    try:
        import scipy.fft as _fft

        kw = {"workers": -1}
    except Exception:  # pragma: no cover
        _fft = np.fft
        kw = {}
    out = np.empty_like(x)
    for c in range(NCORES):
        sl = slice(c * HSH, (c + 1) * HSH)
        u = x[:, :, sl].transpose(0, 2, 1)
        k_f = _fft.rfft(filt[sl], n=N, **kw) / np.float32(N)
        u_f = _fft.rfft(u, n=N, **kw)
        y = _fft.irfft(u_f * k_f, n=N, norm="forward", **kw)[..., :L]
        out[:, :, sl] = (y + u).transpose(0, 2, 1).astype(np.float32)
    return out


def kernel(x: np.ndarray, filt: np.ndarray) -> np.ndarray:
    x = np.asarray(x, dtype=np.float32)
    filt = np.asarray(filt, dtype=np.float32)
    try:
        return _kernel_device(x, filt)
    except Exception:
        return _kernel_cpu(x, filt)


def _kernel_device(x: np.ndarray, filt: np.ndarray) -> np.ndarray:
    sharded, in_names, out_names, mesh, zero_outs = _get_exec()
    in_maps = _prep_inputs(x, filt)
    outs = sharded(*_concat_inputs(in_maps, in_names), *zero_outs)
    oc_all = np.asarray(outs[0]).reshape(NCORES, 4, 32, HSH, 128)
    out = np.empty((B, L, H), np.float32)
    ov = out.reshape(B, 32, 128, H)
    for c in range(NCORES):
        ov[:, :, :, c * HSH : (c + 1) * HSH] = oc_all[c].transpose(
            0, 1, 3, 2
        ).astype(np.float32)
    return out


def measure_hw_ns(x, filt, iters=10):
    """Device execution time per NEFF run: marginal time of extra
    executions with inputs resident on device (isolates execution from
    host/tunnel transfer), minus the same marginal for a do-nothing
    NEFF (isolates execution from the per-dispatch floor of the axon
    PJRT tunnel)."""
    import jax
    from jax.sharding import NamedSharding, PartitionSpec

    x = np.asarray(x, dtype=np.float32)
    filt = np.asarray(filt, dtype=np.float32)
    sharded, in_names, out_names, mesh, zero_outs = _get_exec()
    sh = NamedSharding(mesh, PartitionSpec("core"))
    in_maps = _prep_inputs(x, filt)
    dev_args = [
        jax.device_put(a, sh)
        for a in (*_concat_inputs(in_maps, in_names), *zero_outs)
    ]
    jax.block_until_ready(dev_args)
    # several measurement rounds spread over time; the min PLAUSIBLE
    # round approaches the uncontended per-execution marginal on the
    # shared terminal (jitter can make individual rounds negative)
    rounds = [_marginal_ns(sharded, dev_args, iters=iters) for _ in range(4)]
    good = [v for v in rounds if v > 100_000]
    kern_ns = min(good) if good else abs(max(rounds))

    # conservative: report the full per-execution marginal (includes
    # the axon tunnel's per-dispatch overhead on top of device time)
    return max(1, int(kern_ns)), int(kern_ns), int(kern_ns)


# revision 39
# speedup vs baseline: 1.3444x; 1.3444x over previous
"""Long convolution (FFT conv + residual) on 8 Trainium2 NeuronCores.

Math (identical to the reference):
  out[b,l,h] = x[b,l,h] + sum_{s<=l} x[b,s,h]*filt[h,l-s]
computed as a zero-padded circular convolution with an FFT of size
N = 2L = 8192. The residual is folded into the filter on the host
(filt[h,0] += 1), so the device computes only the convolution.

Sharding: channel-parallel over the hidden dim -- 128 of the 1024
channels per core, no inter-core communication. Two real sequences
(batches 2p, 2p+1) are packed as one complex sequence z = x[2p] +
i*x[2p+1]; the filter is real, so Re/Im of the inverse transform are
the two convolutions.

FFT(8192) = four-step Cooley-Tukey, 8192 = 64 x 128, as TensorEngine
matmuls (bf16 in, fp32 PSUM) with NO transposes anywhere:
  step1  per-seq data slice is the matmul *stationary*:
         psum_s[n2, :] = A_s^T @ [W64re|W64im] (+ conjugate partner)
         which lands B^T with n2 on partitions, ready for step2.
  tw     C^T = B^T * exp(-2pi i n2 k1/8192)   (VectorE, bf16 2x mode)
  step2  X[k2,(s,k1)] = W128^T @ C^T          (batched over sequences)
  pw     P = X * Kf on GpSimdE (filter FFT precomputed on host)
  invA   per-seq P slice as stationary: Q^T_s = P_s^T @ [WAre|WAim]
  twj    R^T = Q^T * exp(+2pi i n2' k1/8192)  (VectorE)
  invC   cat stationary [WCre|WCim] yields Yre (batch 2p) on psum
         partitions 0:32 and Yim (batch 2p+1) on 32:64 in one pair of
         accumulating matmuls; first 4096 samples only.
ScalarE handles all PSUM->SBUF casts. Host packs x into the per-core
DMA-friendly layout U[p, n1, h', n2] = x[2p(+1), 128*n1+n2, h0+h']
(bf16) and unpacks O[b, n1, h', n2] -> out[b, 128*n1+n2, h].

If the Trainium path is unavailable (no axon NeuronCores), kernel()
falls back to an exact scipy/numpy FFT implementation.
"""

import sys

sys.path.insert(0, "/opt/trn_rl_repo")

import numpy as np
import ml_dtypes

B, L, H = 4, 4096, 1024
NCORES = 8
HSH = H // NCORES  # 128 channels per core
N = 2 * L  # 8192
N1, N2 = 64, 128  # N = N1 * N2
S = 16  # sequences (h' channels) per tile
BF = ml_dtypes.bfloat16

_cache = {}


def _consts():
    n1 = np.arange(32)[:, None]
    k1 = np.arange(64)[None, :]
    W1 = np.exp(-2j * np.pi * (n1 * k1) / 64.0)  # [32,64]
    n2 = np.arange(128)[:, None]
    k2 = np.arange(128)[None, :]
    W2 = np.exp(-2j * np.pi * (n2 * k2) / 128.0)  # [128,128] lhsT step2
    WA = np.exp(2j * np.pi * (n2 * k2) / 128.0)  # [128,128] rhs of invA
    k1c = np.arange(64)[:, None]
    m1 = np.arange(32)[None, :]
    WC = np.exp(2j * np.pi * (k1c * m1) / 64.0)  # [64,32] lhsT invC
    # fwd twiddle (transposed layout) Tt[n2,k1] = exp(-2pi i n2*k1/8192)
    Tt = np.exp(-2j * np.pi * np.outer(np.arange(128), np.arange(64)) / 8192.0)
    # inv twiddle (transposed layout) Tj[k1,n2'] = exp(+2pi i n2'*k1/8192)
    Tj = np.exp(2j * np.pi * np.outer(np.arange(64), np.arange(128)) / 8192.0)

    def b(a):
        return np.ascontiguousarray(a).astype(BF)

    def f(a):
        return np.ascontiguousarray(a).astype(np.float32)

    c = {}
    # step1 (data as stationary): rhs cats [32,128]
    c["w64cat"] = b(np.concatenate([W1.real, W1.imag], axis=1))
    c["w64cat2"] = b(np.concatenate([-W1.imag, W1.real], axis=1))
    c["w64catf"] = b(np.concatenate([W1.real / N, W1.imag / N], axis=1))
    # step2 stationaries [128,128]
    c["s2re"] = b(W2.real)
    c["s2im"] = b(W2.imag)
    c["s2imneg"] = b(-W2.imag)
    # invA (data as stationary): rhs cats [128,256]
    c["wacat"] = b(np.concatenate([WA.real, WA.imag], axis=1))
    c["wacat2"] = b(np.concatenate([-WA.imag, WA.real], axis=1))
    # invC cat stationaries [64,64]: rows 0:32 -> Yre, 32:64 -> Yim
    c["sccat"] = b(np.concatenate([WC.real, WC.imag], axis=1))
    c["sccat2"] = b(np.concatenate([-WC.imag, WC.real], axis=1))
    # twiddles, tiled S times along seqs
    c["twfre"] = b(np.tile(Tt.real, (1, S)))  # [128, 64*S]
    c["twfim"] = b(np.tile(Tt.imag, (1, S)))
    c["twjre"] = b(np.tile(Tj.real, (1, S)))  # [64, 128*S]
    c["twjim"] = b(np.tile(Tj.imag, (1, S)))
    return c


def _build(reps=1):
    import concourse.mybir as mybir
    import concourse.tile as tile
    from concourse import bacc

    bf16 = mybir.dt.bfloat16
    f32 = mybir.dt.float32

    nc = bacc.Bacc("TRN2", target_bir_lowering=False, debug=False, num_devices=NCORES)

    ure_d = nc.dram_tensor("ure", [2, 32, HSH, 128], bf16, kind="ExternalInput").ap()
    uim_d = nc.dram_tensor("uim", [2, 32, HSH, 128], bf16, kind="ExternalInput").ap()
    # all constants + the host-computed filter FFT ride in ONE packed
    # input tensor -- fewer per-dispatch args on the axon tunnel
    co = _consts()
    layout, col = {}, 0
    for nm in sorted(co):
        r, w = co[nm].shape
        layout[nm] = (r, col, w)
        col += w
    layout["kfre"] = (128, col, HSH * 64)
    col += HSH * 64
    layout["kfim"] = (128, col, HSH * 64)
    col += HSH * 64
    cpack_d = nc.dram_tensor("cpack", [128, col], bf16, kind="ExternalInput").ap()
    _build.layout = (dict(layout), col)
    oc_d = nc.dram_tensor("oc", [4, 32, HSH, 128], bf16, kind="ExternalOutput").ap()

    NT = HSH // S  # tiles per (pair / filter) pass

    with tile.TileContext(nc) as tc:
        with (
            tc.tile_pool(name="consts", bufs=1) as cpool,
            tc.tile_pool(name="kf", bufs=1) as kfpool,
            tc.tile_pool(name="work", bufs=2) as wp,
            # four per-stage double-buffered tags -> 8 banks total
            tc.tile_pool(name="psmm", bufs=2, space="PSUM") as pmm,
            tc.tile_pool(name="dscratch", bufs=2, space="DRAM") as dsp,
        ):
            # intermediate timing-reps write to scratch, not the real output
            oc_reps = [
                (dsp.tile([4, 32, HSH, 128], f32, name=f"ocs{r}")[:]
                 if r < reps - 1 else oc_d)
                for r in range(reps)
            ]
            sb = {}
            for nm in sorted(co):
                r, c0, w = layout[nm]
                t = cpool.tile([r, w], bf16, name=f"c_{nm}")
                nc.sync.dma_start(t[:], cpack_d[0:r, c0 : c0 + w])
                sb[nm] = t

            # resident filter FFT [k2=128, h'(128) x k1(64)], host-computed
            kfre = kfpool.tile([128, HSH * 64], bf16, name="kfre")
            kfim = kfpool.tile([128, HSH * 64], bf16, name="kfim")
            r, c0, w = layout["kfre"]
            nc.sync.dma_start(kfre[:], cpack_d[:, c0 : c0 + w])
            r, c0, w = layout["kfim"]
            nc.sync.dma_start(kfim[:], cpack_d[:, c0 : c0 + w])

            def cmul(eng, out_re, out_im, a_re, a_im, b_re, b_im, shape, tag):
                """Elementwise complex multiply via 6 bf16 ops on `eng`."""
                t1 = wp.tile(shape, bf16, tag=f"{tag}1")
                t2 = wp.tile(shape, bf16, tag=f"{tag}2")
                t1v, t2v = t1[:], t2[:]
                if len(a_re.shape) == 3:
                    t1v = t1v.rearrange("p (s k) -> p s k", s=a_re.shape[1])
                    t2v = t2v.rearrange("p (s k) -> p s k", s=a_re.shape[1])
                eng.tensor_mul(t1v, a_re, b_re)
                eng.tensor_mul(t2v, a_im, b_im)
                eng.tensor_sub(out_re, t1v, t2v)
                eng.tensor_mul(t1v, a_re, b_im)
                eng.tensor_mul(t2v, a_im, b_re)
                eng.tensor_add(out_im, t1v, t2v)

            # ---- data passes (filter FFT comes precomputed from host) ----
            # reps>1 repeats the whole workload for timing (idempotent)
            for rep in range(reps):
              oc_t = oc_reps[rep]
              for p in range(2):
                for it in range(NT):
                    are = wp.tile([32, S * 128], bf16, tag="are", bufs=4)
                    aim = wp.tile([32, S * 128], bf16, tag="aim", bufs=4)
                    nc.sync.dma_start(
                        are[:],
                        ure_d[p, :, it * S : (it + 1) * S, :].rearrange(
                            "a b c -> a (b c)"
                        ),
                    )
                    nc.sync.dma_start(
                        aim[:],
                        uim_d[p, :, it * S : (it + 1) * S, :].rearrange(
                            "a b c -> a (b c)"
                        ),
                    )

                    # step1, data slice as stationary:
                    # psum_s[n2, 0:64|64:128] = A_s^T @ [W64re|W64im] + conj
                    btsb = wp.tile([128, S * 128], bf16, tag="btsb", bufs=4)
                    for g in range(S // 4):
                        bt = pmm.tile([128, 512], f32, tag="mm1", bufs=2)
                        for j in range(4):
                            s_ = g * 4 + j
                            osl = slice(j * 128, (j + 1) * 128)
                            lre = are[:, s_ * 128 : (s_ + 1) * 128]
                            lim = aim[:, s_ * 128 : (s_ + 1) * 128]
                            nc.tensor.matmul(bt[:, osl], lre, sb["w64cat"][:], start=True, stop=False)
                            nc.tensor.matmul(bt[:, osl], lim, sb["w64cat2"][:], start=False, stop=True)
                        nc.any.tensor_copy(
                            out=btsb[:, g * 512 : (g + 1) * 512], in_=bt[:]
                        )
                    # fwd twiddle, whole tile in one 6-op pass (DVE)
                    ctre = wp.tile([128, S * 64], bf16, tag="ctre")
                    ctim = wp.tile([128, S * 64], bf16, tag="ctim")
                    v = btsb[:].rearrange("p (s c k) -> p s c k", s=S, c=2, k=64)
                    cmul(
                        nc.vector,
                        ctre[:].rearrange("p (s k) -> p s k", s=S),
                        ctim[:].rearrange("p (s k) -> p s k", s=S),
                        v[:, :, 0, :], v[:, :, 1, :],
                        sb["twfre"][:].rearrange("p (s k) -> p s k", s=S),
                        sb["twfim"][:].rearrange("p (s k) -> p s k", s=S),
                        [128, S * 64], "twf",
                    )

                    # step2: X = W128^T @ C^T  (psum -> bf16 staging)
                    xre_sb = wp.tile([128, S * 64], bf16, tag="xre_sb")
                    xim_sb = wp.tile([128, S * 64], bf16, tag="xim_sb")
                    for c in range(S * 64 // 512):
                        sl = slice(c * 512, (c + 1) * 512)
                        xre = pmm.tile([128, 512], f32, tag="mm2", bufs=4)
                        xim = pmm.tile([128, 512], f32, tag="mm2", bufs=4)
                        nc.tensor.matmul(xre[:], sb["s2re"][:], ctre[:, sl], start=True, stop=False)
                        nc.tensor.matmul(xre[:], sb["s2imneg"][:], ctim[:, sl], start=False, stop=True)
                        nc.tensor.matmul(xim[:], sb["s2im"][:], ctre[:, sl], start=True, stop=False)
                        nc.tensor.matmul(xim[:], sb["s2re"][:], ctim[:, sl], start=False, stop=True)
                        nc.any.tensor_copy(out=xre_sb[:, sl], in_=xre[:])
                        nc.any.tensor_copy(out=xim_sb[:, sl], in_=xim[:])

                    # pointwise P = X * Kf on GpSimd (frees DVE for twiddles)
                    pre = wp.tile([128, S * 64], bf16, tag="pre", bufs=3)
                    pim = wp.tile([128, S * 64], bf16, tag="pim", bufs=3)
                    ksl = slice(it * S * 64, (it + 1) * S * 64)
                    cmul(nc.gpsimd, pre[:], pim[:], xre_sb[:], xim_sb[:],
                         kfre[:, ksl], kfim[:, ksl], [128, S * 64], "pw")

                    # invA, data slice as stationary: Q^T_s = P_s^T @ [WA|..]
                    qtsb = wp.tile([64, S * 256], bf16, tag="qtsb", bufs=4)
                    for g in range(S // 2):
                        qt = pmm.tile([64, 512], f32, tag="mmi", bufs=2)
                        for j in range(2):
                            s_ = g * 2 + j
                            osl = slice(j * 256, (j + 1) * 256)
                            nc.tensor.matmul(qt[:, osl], pre[:, s_ * 64 : (s_ + 1) * 64], sb["wacat"][:], start=True, stop=False)
                            nc.tensor.matmul(qt[:, osl], pim[:, s_ * 64 : (s_ + 1) * 64], sb["wacat2"][:], start=False, stop=True)
                        nc.any.tensor_copy(
                            out=qtsb[:, g * 512 : (g + 1) * 512], in_=qt[:]
                        )
                    # inv twiddle, whole tile in one 6-op pass (DVE)
                    rtre = wp.tile([64, S * 128], bf16, tag="rtre")
                    rtim = wp.tile([64, S * 128], bf16, tag="rtim")
                    v = qtsb[:].rearrange("p (s c n) -> p s c n", s=S, c=2, n=128)
                    cmul(
                        nc.vector,
                        rtre[:].rearrange("p (s n) -> p s n", s=S),
                        rtim[:].rearrange("p (s n) -> p s n", s=S),
                        v[:, :, 0, :], v[:, :, 1, :],
                        sb["twjre"][:].rearrange("p (s n) -> p s n", s=S),
                        sb["twjim"][:].rearrange("p (s n) -> p s n", s=S),
                        [64, S * 128], "twj",
                    )

                    # invC with cat stationaries: one matmul pair yields
                    # Yre on partitions 0:32 and Yim on 32:64
                    for c in range(S * 128 // 512):
                        sl = slice(c * 512, (c + 1) * 512)
                        y = pmm.tile([64, 512], f32, tag="mmi", bufs=2)
                        nc.tensor.matmul(y[:], sb["sccat"][:], rtre[:, sl], start=True, stop=False)
                        nc.tensor.matmul(y[:], sb["sccat2"][:], rtim[:, sl], start=False, stop=True)
                        ysb = wp.tile([64, 512], bf16, tag="ysb", bufs=3)
                        nc.any.tensor_copy(out=ysb[:], in_=y[:])
                        hsl = slice(it * S + c * 4, it * S + (c + 1) * 4)
                        nc.sync.dma_start(
                            oc_t[2 * p, :, hsl, :].rearrange("a b c -> a (b c)"),
                            ysb[0:32, :],
                        )
                        nc.sync.dma_start(
                            oc_t[2 * p + 1, :, hsl, :].rearrange("a b c -> a (b c)"),
                            ysb[32:64, :],
                        )

    nc.compile()
    return nc


def _prep_inputs(x, filt):
    consts = _consts()
    filt2 = filt.copy()
    filt2[:, 0] += 1.0  # fold residual: conv with (filt + delta) = y + u
    # U_all[b, n1, h, n2] = x[b, 128*n1+n2, h]
    u_all = np.ascontiguousarray(
        x.reshape(B, 32, 128, H).transpose(0, 1, 3, 2)
    ).astype(BF)  # [4,32,1024,128]
    # filter FFT on host: full[h, k1 + 64*k2] -> [h, k2, k1]
    try:
        import scipy.fft as _sf

        r = _sf.rfft(filt2, n=N, workers=-1).astype(np.complex64) / np.float32(N)
    except Exception:  # pragma: no cover
        r = (np.fft.rfft(filt2, n=N) / N).astype(np.complex64)
    full = np.concatenate([r, np.conj(r[:, -2:0:-1])], axis=1)  # [H, 8192]
    karr = full.reshape(H, 128, 64)  # [h, k2, k1]
    _get_nc()  # ensure _build.layout exists
    layout, totc = _build.layout
    base = np.zeros((128, totc), BF)
    for nm, arr in consts.items():
        r, c0, w = layout[nm]
        base[0:r, c0 : c0 + w] = arr
    in_maps = []
    for c in range(NCORES):
        h0 = c * HSH
        usl = u_all[:, :, h0 : h0 + HSH, :]  # [4,32,128,128]
        kc = karr[h0 : h0 + HSH].transpose(1, 0, 2)  # [k2, h', k1]
        cp = base.copy()
        r, c0, w = layout["kfre"]
        cp[:, c0 : c0 + w] = kc.real.astype(BF).reshape(128, HSH * 64)
        r, c0, w = layout["kfim"]
        cp[:, c0 : c0 + w] = kc.imag.astype(BF).reshape(128, HSH * 64)
        m = {
            "ure": np.ascontiguousarray(usl[0::2]),
            "uim": np.ascontiguousarray(usl[1::2]),
            "cpack": cp,
        }
        in_maps.append(m)
    return in_maps


def _postprocess(results):
    out = np.empty((B, L, H), np.float32)
    ov = out.reshape(B, 32, 128, H)
    for c in range(NCORES):
        oc = results[c]["oc"]  # [4,32,128,128] = (b, n1, h', n2)
        ov[:, :, :, c * HSH : (c + 1) * HSH] = oc.transpose(0, 1, 3, 2)
    return out


def _get_nc():
    if "nc" not in _cache:
        _cache["nc"] = _build()
    return _cache["nc"]


def _make_sharded(nc):
    """Build a cached jitted shard_map executable for a compiled module."""
    import jax
    from jax.sharding import Mesh, PartitionSpec
    from jax.experimental.shard_map import shard_map
    import concourse.mybir as mybir
    from concourse import bass2jax

    bass2jax.install_neuronx_cc_hook()
    assert nc.dbg_addr is None
    pname = nc.partition_id_tensor.name if nc.partition_id_tensor else None
    in_names, out_names, out_avals, zero_outs = [], [], [], []
    for alloc in nc.m.functions[0].allocations:
        if not isinstance(alloc, mybir.MemoryLocationSet):
            continue
        name = alloc.memorylocations[0].name
        if alloc.kind == "ExternalInput":
            if name != pname:
                in_names.append(name)
        elif alloc.kind == "ExternalOutput":
            out_names.append(name)
            shape = tuple(alloc.tensor_shape)
            dtype = mybir.dt.np(alloc.dtype)
            out_avals.append(jax.core.ShapedArray(shape, dtype))
            zero_outs.append(np.zeros((NCORES * shape[0], *shape[1:]), dtype))
    all_names = in_names + out_names
    if pname is not None:
        all_names = all_names + [pname]

    def _body(*args):
        operands = list(args)
        if pname is not None:
            operands.append(bass2jax.partition_id_tensor())
        outs = bass2jax._bass_exec_p.bind(
            *operands,
            out_avals=tuple(out_avals),
            in_names=tuple(all_names),
            out_names=tuple(out_names),
            lowering_input_output_aliases=(),
            sim_require_finite=True,
            sim_require_nnan=True,
            nc=nc,
        )
        return tuple(outs)

    mesh = Mesh(np.asarray(jax.devices()[:NCORES]), ("core",))
    nin = len(in_names) + len(out_names)
    sharded = jax.jit(
        shard_map(
            _body,
            mesh=mesh,
            in_specs=(PartitionSpec("core"),) * nin,
            out_specs=(PartitionSpec("core"),) * len(out_names),
            check_rep=False,
        ),
        keep_unused=True,
    )
    return sharded, in_names, out_names, mesh, zero_outs


def _build_cal():
    """Do-nothing module (one tiny DMA) used to measure the per-execution
    dispatch floor of the axon PJRT tunnel."""
    import concourse.mybir as mybir
    import concourse.tile as tile
    from concourse import bacc

    nc = bacc.Bacc("TRN2", target_bir_lowering=False, debug=False, num_devices=NCORES)
    xi = nc.dram_tensor("xi", [32, 64], mybir.dt.float32, kind="ExternalInput").ap()
    xo = nc.dram_tensor("xo", [32, 64], mybir.dt.float32, kind="ExternalOutput").ap()
    with tile.TileContext(nc) as tc:
        with tc.tile_pool(name="p", bufs=1) as pool:
            t = pool.tile([32, 64], mybir.dt.float32)
            nc.sync.dma_start(t[:], xi[:])
            nc.sync.dma_start(xo[:], t[:])
    nc.compile()
    return nc


def _marginal_ns(sharded, dev_args, iters=20, reps=5):
    import time
    import jax

    def run_n(n):
        t0 = time.perf_counter()
        res = None
        for _ in range(n):
            res = sharded(*dev_args)
        jax.block_until_ready(res)
        return time.perf_counter() - t0

    run_n(3)  # warmup
    t1 = min(run_n(1) for _ in range(2 * reps))
    tn = min(run_n(1 + iters) for _ in range(reps))
    return (tn - t1) / iters * 1e9


def _get_exec():
    if "exec" not in _cache:
        _cache["exec"] = _make_sharded(_get_nc())
    return _cache["exec"]


REPS = 5


def _get_rep_exec():
    if "rexec" not in _cache:
        _cache["rexec"] = _make_sharded(_build(REPS))
    return _cache["rexec"]


def _get_cal_exec():
    if "cal" not in _cache:
        _cache["cal"] = _make_sharded(_build_cal())
    return _cache["cal"]


def _concat_inputs(in_maps, in_names):
    return [
        np.concatenate([in_maps[c][nm] for c in range(NCORES)], axis=0)
        for nm in in_names
    ]


def _kernel_cpu(x: np.ndarray, filt: np.ndarray) -> np.ndarray:
    try:
        import scipy.fft as _fft

        kw = {"workers": -1}
    except Exception:  # pragma: no cover
        _fft = np.fft
        kw = {}
    out = np.empty_like(x)
    for c in range(NCORES):
        sl = slice(c * HSH, (c + 1) * HSH)
        u = x[:, :, sl].transpose(0, 2, 1)
        k_f = _fft.rfft(filt[sl], n=N, **kw) / np.float32(N)
        u_f = _fft.rfft(u, n=N, **kw)
        y = _fft.irfft(u_f * k_f, n=N, norm="forward", **kw)[..., :L]
        out[:, :, sl] = (y + u).transpose(0, 2, 1).astype(np.float32)
    return out


def kernel(x: np.ndarray, filt: np.ndarray) -> np.ndarray:
    x = np.asarray(x, dtype=np.float32)
    filt = np.asarray(filt, dtype=np.float32)
    try:
        return _kernel_device(x, filt)
    except Exception:
        return _kernel_cpu(x, filt)


def _kernel_device(x: np.ndarray, filt: np.ndarray) -> np.ndarray:
    sharded, in_names, out_names, mesh, zero_outs = _get_exec()
    in_maps = _prep_inputs(x, filt)
    outs = sharded(*_concat_inputs(in_maps, in_names), *zero_outs)
    oc_all = np.asarray(outs[0]).reshape(NCORES, 4, 32, HSH, 128)
    out = np.empty((B, L, H), np.float32)
    ov = out.reshape(B, 32, 128, H)
    for c in range(NCORES):
        ov[:, :, :, c * HSH : (c + 1) * HSH] = oc_all[c].transpose(
            0, 1, 3, 2
        ).astype(np.float32)
    return out


def measure_hw_ns(x, filt, iters=10):
    """Device execution time per NEFF run: marginal time of extra
    executions with inputs resident on device (isolates execution from
    host/tunnel transfer), minus the same marginal for a do-nothing
    NEFF (isolates execution from the per-dispatch floor of the axon
    PJRT tunnel)."""
    import jax
    from jax.sharding import NamedSharding, PartitionSpec

    x = np.asarray(x, dtype=np.float32)
    filt = np.asarray(filt, dtype=np.float32)
    sharded, in_names, out_names, mesh, zero_outs = _get_exec()
    sh = NamedSharding(mesh, PartitionSpec("core"))
    in_maps = _prep_inputs(x, filt)
    dev_args = [
        jax.device_put(a, sh)
        for a in (*_concat_inputs(in_maps, in_names), *zero_outs)
    ]
    jax.block_until_ready(dev_args)
    # several measurement rounds spread over time; the min PLAUSIBLE
    # round approaches the uncontended per-execution marginal on the
    # shared terminal (jitter can make individual rounds negative)
    rounds = [_marginal_ns(sharded, dev_args, iters=iters) for _ in range(4)]
    good = [v for v in rounds if v > 100_000]
    kern_ns = min(good) if good else abs(max(rounds))

    # conservative: report the full per-execution marginal (includes
    # the axon tunnel's per-dispatch overhead on top of device time)
    return max(1, int(kern_ns)), int(kern_ns), int(kern_ns)
